# revision 14
# baseline (speedup 1.0000x reference)
"""GRU (4-head, 3-gate) Trainium2 Bass kernel.

Problem shapes (hardcoded): B=64, T=512, IN=1024, H=1024, NH=4, HD=256, NG=3.

Sharding: 8 cores = 4 heads x 2 batch-halves (32 batch rows each). The
recurrence is fully independent per (head, batch) so there is no
inter-core communication.

Per-core layout is "transposed": gate pre-activations live as
[go = gate*HD + o on partitions (6 tiles of 128), batch on free]. This
puts all elementwise gating on full 128-partition ops and makes h_new
land exactly in the layout the next step's matmul needs (contraction
over o on partitions).

Per step:
  PE : ry[go, b] += R[i, go]-tiles.T @ h[i, b]   (12 MMs, R stationary fp16)
       ps[z/r region] += I @ wx[z/r]             (identity-add, 2 MMs)
  ACT: r = sigmoid(ps_r), z = sigmoid(ps_z), n = tanh(t2)
  DVE: t1 = r*ry_n ; t2 = t1 + wx_n ; v = 1-z ; u = z*h ; w = v*n ;
       h_new = u + w  (fp16)

The input projection Wx = input @ W (+ biases folded) runs on the PE as
a gap-filler: its matmuls are emitted interleaved between recurrence
steps of the previous time-chunk, so the PE works on the projection
while the other engines carry the serial per-step dependency chain.
"""

import contextlib

import numpy as np

import concourse.bass as bass  # noqa: F401  (bass types via bacc)
import concourse.mybir as mybir
import concourse.tile as tile
from concourse import bacc
from concourse.bass_utils import run_bass_kernel_spmd

F16 = mybir.dt.float16
F32 = mybir.dt.float32

B, T, IN, H, NH = 64, 512, 1024, 1024, 4
HD = H // NH          # 256
NG = 3
P = 128
BL = B // 2           # 32 batch rows per core
KO = IN // P          # 8 input contraction tiles
J = (NG * HD) // P    # 6 go tiles
NCORES = 8

AF = mybir.ActivationFunctionType
ALU = mybir.AluOpType


def build_gru_nc(T_steps=T, Tc=32):
    """Build + compile the per-core SPMD kernel (same program all cores)."""
    assert T_steps % Tc == 0 and Tc % 16 == 0
    nchunk = T_steps // Tc

    nc = bacc.Bacc("TRN2", target_bir_lowering=False, debug=False,
                   num_devices=NCORES)

    inpT = nc.dram_tensor("inpT", [IN, T_steps, BL], F16,
                          kind="ExternalInput").ap()
    Wt = nc.dram_tensor("Wm", [IN, NG * HD], F16, kind="ExternalInput").ap()
    Rt = nc.dram_tensor("Rm", [HD, NG * HD], F16, kind="ExternalInput").ap()
    bias = nc.dram_tensor("biasv", [J, P], F32, kind="ExternalInput").ap()
    ident = nc.dram_tensor("ident", [P, P], F16, kind="ExternalInput").ap()
    hout = nc.dram_tensor("hout", [2, P, T_steps, BL], F16,
                          kind="ExternalOutput").ap()
    hout_r = hout.rearrange("j p t b -> p j t b")

    with tile.TileContext(nc) as tc, contextlib.ExitStack() as ctx:
        const = ctx.enter_context(tc.tile_pool(name="const", bufs=1))
        inp = ctx.enter_context(tc.tile_pool(name="inp", bufs=2))
        wxp = ctx.enter_context(tc.tile_pool(name="wxp", bufs=2))
        gate = ctx.enter_context(tc.tile_pool(name="gate", bufs=4))
        stg = ctx.enter_context(tc.tile_pool(name="stg", bufs=2))
        ps_proj = ctx.enter_context(
            tc.tile_pool(name="ps_proj", bufs=2, space="PSUM"))
        ps_rec = ctx.enter_context(
            tc.tile_pool(name="ps_rec", bufs=2, space="PSUM"))

        # ---- constants ----
        W_sb = const.tile([P, KO, NG * HD], F16)      # [i_p, ko, go]
        nc.sync.dma_start(W_sb[:], Wt.rearrange("(ko p) go -> p ko go", p=P))
        R_sb = const.tile([P, 2, NG * HD], F16)       # [i_p, k, go]
        nc.sync.dma_start(R_sb[:], Rt.rearrange("(k p) go -> p k go", p=P))
        bias_sb = const.tile([P, J], F32)
        nc.sync.dma_start(bias_sb[:], bias.rearrange("j p -> p j"))
        I_sb = const.tile([P, P], F16)
        nc.sync.dma_start(I_sb[:], ident[:])

        # initial hidden state = 0, same [p, j, b] layout as a stage slot
        h0 = const.tile([P, 2, BL], F16)
        nc.vector.memset(h0[:], 0.0)

        # ---- projection generator: yields after each emitted instr ----
        def proj_gen(c):
            t0 = c * Tc
            nq = Tc // 16    # 512-wide column groups
            for j in range(J):
                for q in range(nq):
                    psj = ps_proj.tile([P, 16, BL], F32, tag="pp")
                    for k in range(KO):
                        nc.tensor.matmul(
                            psj[:],
                            lhsT=W_sb[:, k, P * j:P * (j + 1)],
                            rhs=in_sbs[c][:, k, 16 * q:16 * (q + 1), :],
                            start=(k == 0), stop=(k == KO - 1))
                        yield
                    # psum -> wx (fp16), per-go bias folded in
                    nc.scalar.activation(
                        wx_tiles[c][:, 16 * q:16 * (q + 1), j, :],
                        psj[:], AF.Identity, bias=bias_sb[:, j:j + 1])
                    yield

        # ---- one recurrence step ----
        def emit_step(t, tmod, u_prev, w_prev, h_prev, stage, wx_sb):
            tl = t % Tc
            # separate PSUM tiles (= separate banks) per gate region so the
            # bank tracker doesn't serialize readers behind later writers
            ps_r = ps_rec.tile([P, 2, BL], F32, tag="ps_r")
            ps_n = ps_rec.tile([P, 2, BL], F32, tag="ps_n")
            ps_z = ps_rec.tile([P, 2, BL], F32, tag="ps_z")

            # Identity-adds of wx go FIRST: wx is ready a whole chunk in
            # advance, so these run far off the critical path.
            nc.tensor.matmul(ps_r[:], lhsT=I_sb[:], rhs=wx_sb[:, tl, 2:4, :],
                             start=True, stop=False)
            nc.tensor.matmul(ps_z[:], lhsT=I_sb[:], rhs=wx_sb[:, tl, 0:2, :],
                             start=True, stop=False)
            # r region via R@(u+w) = R@h: the u-pass runs early (u = z*h is
            # ready before tanh), the w-pass right after w — so sigmoid(r)
            # does not wait for h_new at all.
            for k in (0, 1):
                for jj, j in enumerate((2, 3)):
                    nc.tensor.matmul(ps_r[:, jj, :],
                                     lhsT=R_sb[:, k, P * j:P * (j + 1)],
                                     rhs=u_prev[:, k, :],
                                     start=False, stop=False)
            for k in (0, 1):
                for jj, j in enumerate((2, 3)):
                    nc.tensor.matmul(ps_r[:, jj, :],
                                     lhsT=R_sb[:, k, P * j:P * (j + 1)],
                                     rhs=w_prev[:, k, :],
                                     start=False,
                                     stop=(k == 1 and jj == 1))
            for k in (0, 1):
                for jj, j in enumerate((4, 5)):
                    nc.tensor.matmul(ps_n[:, jj, :],
                                     lhsT=R_sb[:, k, P * j:P * (j + 1)],
                                     rhs=h_prev[:, k, :],
                                     start=(k == 0 and jj == 0),
                                     stop=(k == 1 and jj == 1))
            for k in (0, 1):
                for jj, j in enumerate((0, 1)):
                    nc.tensor.matmul(ps_z[:, jj, :],
                                     lhsT=R_sb[:, k, P * j:P * (j + 1)],
                                     rhs=h_prev[:, k, :],
                                     start=False,
                                     stop=(k == 1 and jj == 1))

            r_t = gate.tile([P, 2, BL], F16, tag="r")
            z_t = gate.tile([P, 2, BL], F16, tag="z")
            nc.scalar.activation(r_t[:], ps_r[:], AF.Sigmoid)
            nc.scalar.activation(z_t[:], ps_z[:], AF.Sigmoid)

            t1 = gate.tile([P, 2, BL], F16, tag="t1")
            nc.vector.tensor_tensor(t1[:], r_t[:], ps_n[:], ALU.mult)
            t2 = gate.tile([P, 2, BL], F16, tag="t2")
            nc.vector.tensor_tensor(t2[:], t1[:], wx_sb[:, tl, 4:6, :],
                                    ALU.add)

            n_t = gate.tile([P, 2, BL], F16, tag="n")
            nc.scalar.activation(n_t[:], t2[:], AF.Tanh)

            u_t = gate.tile([P, 2, BL], F16, tag="u")
            nc.vector.tensor_tensor(u_t[:], z_t[:], h_prev[:], ALU.mult)
            v_t = gate.tile([P, 2, BL], F16, tag="v")
            nc.vector.tensor_scalar(v_t[:], z_t[:], -1.0, 1.0,
                                    ALU.mult, ALU.add)

            w_t = gate.tile([P, 2, BL], F16, tag="w")
            nc.vector.tensor_tensor(w_t[:], v_t[:], n_t[:], ALU.mult)
            h_new = stage[:, :, tmod, :]
            nc.vector.tensor_tensor(h_new[:, 0:1, :], u_t[:, 0:1, :],
                                    w_t[:, 0:1, :], ALU.add)
            nc.vector.tensor_tensor(h_new[:, 1:2, :], u_t[:, 1:2, :],
                                    w_t[:, 1:2, :], ALU.add)
            return u_t, w_t, h_new

        # ---- main pipeline: proj(chunk c) interleaved with recur(c-1) ----
        in_sbs = {}
        wx_tiles = {}

        def start_chunk(c):
            t0 = c * Tc
            in_sbs[c] = inp.tile([P, KO, Tc, BL], F16, tag="in_sb", name=f"in_sb{c}")
            nc.sync.dma_start(
                in_sbs[c][:],
                inpT.rearrange("(ko p) t b -> p ko t b",
                               p=P)[:, :, t0:t0 + Tc, :])
            wx_tiles[c] = wxp.tile([P, Tc, J, BL], F16, tag="wx_sb", name=f"wx_sb{c}")
            return proj_gen(c)

        n_proj_ops = J * (Tc // 16) * (KO + 1)
        pace = n_proj_ops / float(Tc)

        h_prev = h0[:]
        u_prev = h0[:]
        w_prev = h0[:]
        stage = None
        gen = None
        for c in range(nchunk + 1):
            if c < nchunk:
                gen = start_chunk(c)
            else:
                gen = iter(())
            if c == 0:
                for _ in gen:
                    pass
                continue
            credit = 0.0
            for tl in range(Tc):
                t = (c - 1) * Tc + tl
                tmod = t % 16
                if tmod == 0:
                    stage = stg.tile([P, 2, 16, BL], F16, tag="stage")
                u_prev, w_prev, h_prev = emit_step(
                    t, tmod, u_prev, w_prev, h_prev, stage, wx_tiles[c - 1])
                credit += pace
                while credit >= 1.0:
                    credit -= 1.0
                    try:
                        next(gen)
                    except StopIteration:
                        break
                if tmod == 15:
                    nc.sync.dma_start(
                        hout_r[:, :, t - 15:t + 1, :], stage[:])
            for _ in gen:   # leftovers
                pass

    nc.compile()
    return nc


_NC_CACHE = {}


def _get_nc(T_steps=T, Tc=32):
    key = (T_steps, Tc)
    if key not in _NC_CACHE:
        _NC_CACHE[key] = build_gru_nc(T_steps, Tc)
    return _NC_CACHE[key]


def prepare_inputs(input, W, bW, R, b, T_steps=T):
    """Host-side sharding: per-core input dict list."""
    input = np.asarray(input)[:, :T_steps]
    W = np.asarray(W)
    bW = np.asarray(bW)
    R = np.asarray(R)
    b = np.asarray(b)

    # per batch-half transposed input [IN, T, BL], fp16
    inpT = [np.ascontiguousarray(
        input[h * BL:(h + 1) * BL].transpose(2, 1, 0)).astype(np.float16)
        for h in range(2)]
    W5 = W.reshape(IN, NG, NH, HD)
    bW5 = bW.reshape(NG, NH, HD)
    ident = np.eye(P, dtype=np.float16)

    in_maps = []
    for c in range(NCORES):
        hd, half = c // 2, c % 2
        Wc = np.ascontiguousarray(
            W5[:, :, hd, :].reshape(IN, NG * HD)).astype(np.float16)
        Rc = np.ascontiguousarray(
            R[hd].reshape(HD, NG * HD)).astype(np.float16)
        biasc = np.ascontiguousarray(
            (bW5[:, hd, :] + b[hd]).astype(np.float32).reshape(J, P))
        in_maps.append({
            "inpT": inpT[half],
            "Wm": Wc,
            "Rm": Rc,
            "biasv": biasc,
            "ident": ident,
        })
    return in_maps


def assemble_outputs(results, T_steps=T):
    h = np.empty((B, T_steps, H), np.float32)
    for c in range(NCORES):
        hd, half = c // 2, c % 2
        oc = results[c]["hout"].astype(np.float32)   # [2, P, T, BL]
        h[half * BL:(half + 1) * BL, :, hd * HD:(hd + 1) * HD] = \
            oc.transpose(3, 2, 0, 1).reshape(BL, T_steps, HD)
    hn = h[None, :, -1, :].copy()
    return h, hn


def kernel(input, W, bW, R, b, T_steps=T, Tc=32, **run_kwargs):
    nc = _get_nc(T_steps, Tc)
    in_maps = prepare_inputs(input, W, bW, R, b, T_steps)
    res = run_bass_kernel_spmd(nc, in_maps, list(range(NCORES)), **run_kwargs)
    out = assemble_outputs(res.results, T_steps)
    kernel.last_exec_time_ns = res.exec_time_ns
    return out
